# revision 11
# baseline (speedup 1.0000x reference)
"""Distributed Trainium2 kernel for gnn_message_passing (nn_AMN_18004502905276).

Reference computation:
    masked = where(conn > 0.1, conn, 0)          # [64, 64]
    w      = 3.0 * masked.sum(axis=0)            # [64]
    out    = einsum('j,jtn->tn', w, unit_outputs)  # [100, 4096]

Strategy: shard along N (4096 = 8 x 512) so every core computes its own
output slice with zero collectives.  Per core the weighted unit-sum is a
[128,2]^T @ [128,400] matmul: the moving operand stacks two 64-unit groups
on the 128 partitions, the stationary operand is a block-diagonal copy of
w, so each matmul consumes 800 data columns.  w itself is computed on
device (mask + ones-matmul).  float32r gives 1 cycle/column on the PE.
"""

import contextlib
import sys

import numpy as np

sys.path.insert(0, "/opt/trn_rl_repo")

import concourse.bass as bass
import concourse.mybir as mybir
from concourse.bass_utils import run_bass_kernel_spmd

# Problem geometry (hardcoded per the harness contract).
U, T, N = 64, 100, 4096
NCORES = 8
NS = N // NCORES          # 512 output columns per core
FLAT = T * NS             # 51200 flat (t, n) positions per core
TILE_F = 1600             # moving columns per SBUF tile half
NTILES = FLAT // (2 * TILE_F)   # 16
MM_F = 400                # moving columns per matmul (>=256 keeps f32r at 1 cyc/col)
MMS_PER_TILE = TILE_F // MM_F   # 4
G_TOTAL = NTILES * MMS_PER_TILE  # 64 main matmuls
NB = 4                    # x-tile double buffers
F32 = mybir.dt.float32
BF16 = mybir.dt.bfloat16

THRESHOLD = 0.1
STRENGTH = 3.0


def build_nc() -> bass.Bass:
    nc = bass.Bass()

    x_d = nc.declare_dram_parameter("x", [NTILES, 128, TILE_F], F32, isOutput=False)
    conn_d = nc.declare_dram_parameter("conn", [U, U], F32, isOutput=False)
    out_d = nc.declare_dram_parameter("out", [8, NTILES * MM_F], F32, isOutput=True)

    ctx = contextlib.ExitStack()
    with ctx:
        xb = [
            ctx.enter_context(nc.sbuf_tensor(f"xb{i}", [128, TILE_F], BF16))
            for i in range(NB)
        ]
        conn_sb = ctx.enter_context(nc.sbuf_tensor([U, U], F32))
        masked = ctx.enter_context(nc.sbuf_tensor([U, U], F32))
        ones_sb = ctx.enter_context(nc.sbuf_tensor([U, 1], F32))
        s_sb = ctx.enter_context(nc.sbuf_tensor([128, 2], BF16))
        out_sb = ctx.enter_context(nc.sbuf_tensor([128, NTILES * MM_F], F32))
        psum = ctx.enter_context(nc.psum_tensor([128, 4096], F32))

        ctx.enter_context(nc.Block())
        block = nc.cur_block
        dma_c = ctx.enter_context(nc.semaphore("dma_c"))
        dma_x = [
            ctx.enter_context(nc.semaphore(f"dma_x{i}")) for i in range(NTILES)
        ]
        dma_out = ctx.enter_context(nc.semaphore("dma_out"))
        mm_sem = ctx.enter_context(nc.semaphore("mm_sem"))
        ve_sem = ctx.enter_context(nc.semaphore("ve_sem"))
        s_sem = ctx.enter_context(nc.semaphore("s_sem"))
        cpv_sem = ctx.enter_context(nc.semaphore("cpv_sem"))
        cps_sem = ctx.enter_context(nc.semaphore("cps_sem"))

        def psum_slot(g):
            q, bank = g // 16, g % 8
            return psum[32 * q : 32 * q + 2, bank * 512 : bank * 512 + MM_F]

        def out_slot(g):
            q, r = g // 16, g % 16
            return out_sb[32 * q : 32 * q + 2, r * MM_F : (r + 1) * MM_F]

        @block.sync
        def _(sync):
            sync.dma_start(out=conn_sb[:, :], in_=conn_d[:, :]).then_inc(dma_c, 16)
            for q in range(4):
                sync.wait_ge(cpv_sem, 8 * (q + 1))
                sync.wait_ge(cps_sem, 8 * (q + 1))
                sync.dma_start(
                    out=out_d[2 * q : 2 * q + 2, :],
                    in_=out_sb[32 * q : 32 * q + 2, :],
                ).then_inc(dma_out, 16)
            sync.wait_ge(dma_out, 64)

        @block.gpsimd
        def _(gpsimd):
            # SWDGE casts f32 -> bf16 inline during the load
            for k in range(NTILES):
                if k >= NB:
                    # all 4 matmuls of tile k-NB done -> xb[k % NB] is free
                    gpsimd.wait_ge(mm_sem, 2 + 4 * (k - NB) + 4)
                gpsimd.dma_start(out=xb[k % NB][:, :], in_=x_d[k]).then_inc(
                    dma_x[k], 16
                )

        @block.vector
        def _(vector):
            vector.memset(ones_sb[:, :], 1.0).then_inc(ve_sem)
            vector.memset(s_sb[:, :], 0.0).then_inc(ve_sem)
            vector.wait_ge(dma_c, 16)
            # masked = (conn > 0.1) * conn
            vector.scalar_tensor_tensor(
                out=masked[:, :],
                in0=conn_sb[:, :],
                scalar=THRESHOLD,
                in1=conn_sb[:, :],
                op0=mybir.AluOpType.is_gt,
                op1=mybir.AluOpType.mult,
            ).then_inc(ve_sem)
            for g in range(0, G_TOTAL, 2):
                vector.wait_ge(mm_sem, 3 + g)
                vector.tensor_copy(out=out_slot(g), in_=psum_slot(g)).then_inc(cpv_sem)

        @block.scalar
        def _(scalar):
            scalar.wait_ge(mm_sem, 2)
            # S[0:64, 0] = 3 * w ; S[64:128, 1] = 3 * w  (block diagonal)
            scalar.mul(s_sb[0:64, 0:1], psum[0:64, 0:1], STRENGTH).then_inc(s_sem)
            scalar.mul(s_sb[64:128, 1:2], psum[64:128, 0:1], STRENGTH).then_inc(s_sem)
            for g in range(1, G_TOTAL, 2):
                scalar.wait_ge(mm_sem, 3 + g)
                scalar.copy(out_slot(g), psum_slot(g)).then_inc(cps_sem)

        @block.tensor
        def _(tensor):
            tensor.wait_ge(ve_sem, 3)
            # w[j] = sum_i masked[i, j], materialized on partitions 0-63 and 64-127
            tensor.matmul(
                psum[0:64, 0:1], masked[:, :], ones_sb[:, :], start=True, stop=True
            ).then_inc(mm_sem)
            tensor.matmul(
                psum[64:128, 0:1],
                masked[:, :],
                ones_sb[:, :],
                start=True,
                stop=True,
                tile_position=(0, 64),
            ).then_inc(mm_sem)
            tensor.wait_ge(s_sem, 2)
            for k in range(NTILES):
                tensor.wait_ge(dma_x[k], 16)
                for m in range(MMS_PER_TILE):
                    g = 4 * k + m
                    if g >= 8:
                        gg = g - 8  # copy of the matmul that last used this bank
                        if gg % 2 == 0:
                            tensor.wait_ge(cpv_sem, gg // 2 + 1)
                        else:
                            tensor.wait_ge(cps_sem, gg // 2 + 1)
                    q = g // 16
                    tensor.matmul(
                        psum_slot(g),
                        s_sb[:, :],
                        xb[k % NB][:, m * MM_F : (m + 1) * MM_F],
                        start=True,
                        stop=True,
                        tile_position=(0, 32 * q),
                    ).then_inc(mm_sem)

    return nc


def shard_inputs(unit_outputs: np.ndarray, conn: np.ndarray):
    """Full inputs -> per-core in_maps with the tile layout the kernel expects."""
    conn = np.ascontiguousarray(conn, dtype=np.float32)
    in_maps = []
    for c in range(NCORES):
        xc = np.ascontiguousarray(
            unit_outputs[:, :, c * NS : (c + 1) * NS], dtype=np.float32
        ).reshape(U, FLAT)
        # [u, k, h, f] -> [k, (h u), f]
        v = xc.reshape(U, NTILES, 2, TILE_F)
        tiles = np.ascontiguousarray(v.transpose(1, 2, 0, 3)).reshape(
            NTILES, 128, TILE_F
        )
        in_maps.append({"x": tiles, "conn": conn})
    return in_maps


def unshard_output(results) -> np.ndarray:
    """Per-core [8, 6400] outputs -> full [T, N]."""
    final = np.empty((T, N), dtype=np.float32)
    for c in range(NCORES):
        r = np.asarray(results[c]["out"], dtype=np.float32)
        # rows 2q+h, cols (g%16)*400+c ; g = 16q + 4*r1 + r2 ; k = 4q+r1, m = r2
        arr = r.reshape(4, 2, 4, 4, MM_F)  # [q, h, r1, r2, c]
        flat = arr.transpose(0, 2, 1, 3, 4).reshape(FLAT)  # [k, h, m, c] order
        final[:, c * NS : (c + 1) * NS] = flat.reshape(T, NS)
    return final


_NC_CACHE = None


def kernel(unit_outputs: np.ndarray, conn: np.ndarray) -> np.ndarray:
    global _NC_CACHE
    if _NC_CACHE is None:
        _NC_CACHE = build_nc()
    in_maps = shard_inputs(unit_outputs, conn)
    res = run_bass_kernel_spmd(_NC_CACHE, in_maps, core_ids=list(range(NCORES)))
    return unshard_output(res.results)


if __name__ == "__main__":
    rng = np.random.default_rng(0)
    uo = rng.random((U, T, N), dtype=np.float32)
    cn = rng.random((U, U), dtype=np.float32)
    out = kernel(uo, cn)
    w = np.where(cn > THRESHOLD, cn, 0.0).sum(axis=0) * STRENGTH
    ref = np.einsum("j,jtn->tn", w, uo)
    err = np.abs(out - ref).max() / np.abs(ref).max()
    print("rel err:", err)


# revision 14
# speedup vs baseline: 1.0991x; 1.0991x over previous
"""Distributed Trainium2 kernel for gnn_message_passing (nn_AMN_18004502905276).

Reference computation:
    masked = where(conn > 0.1, conn, 0)          # [64, 64]
    w      = 3.0 * masked.sum(axis=0)            # [64]
    out    = einsum('j,jtn->tn', w, unit_outputs)  # [100, 4096]

Strategy: shard along N (4096 = 8 x 512) so every core computes its own
output slice with zero collectives.  Per core the weighted unit-sum is a
[128,2]^T @ [128,400] matmul: the moving operand stacks two 64-unit groups
on the 128 partitions, the stationary operand is a block-diagonal copy of
w, so each matmul consumes 800 data columns.  w itself is computed on
device (mask + ones-matmul).  float32r gives 1 cycle/column on the PE.
"""

import contextlib
import sys

import numpy as np

sys.path.insert(0, "/opt/trn_rl_repo")

import concourse.bass as bass
import concourse.mybir as mybir
from concourse.bass_utils import run_bass_kernel_spmd

# Problem geometry (hardcoded per the harness contract).
U, T, N = 64, 100, 4096
NCORES = 8
NS = N // NCORES          # 512 output columns per core
FLAT = T * NS             # 51200 flat (t, n) positions per core
TILE_F = 1600             # moving columns per SBUF tile half
NTILES = FLAT // (2 * TILE_F)   # 16
MM_F = 400                # moving columns per matmul (>=256 keeps f32r at 1 cyc/col)
MMS_PER_TILE = TILE_F // MM_F   # 4
G_TOTAL = NTILES * MMS_PER_TILE  # 64 main matmuls
NB = 4                    # x-tile double buffers
F32 = mybir.dt.float32
BF16 = mybir.dt.bfloat16

THRESHOLD = 0.1
STRENGTH = 3.0


def build_nc() -> bass.Bass:
    nc = bass.Bass()

    x_d = nc.declare_dram_parameter("x", [NTILES, 128, TILE_F], BF16, isOutput=False)
    conn_d = nc.declare_dram_parameter("conn", [U, U], F32, isOutput=False)
    out_d = nc.declare_dram_parameter("out", [8, NTILES * MM_F], F32, isOutput=True)

    ctx = contextlib.ExitStack()
    with ctx:
        xb = [
            ctx.enter_context(nc.sbuf_tensor(f"xb{i}", [128, TILE_F], BF16))
            for i in range(NB)
        ]
        conn_sb = ctx.enter_context(nc.sbuf_tensor([U, U], F32))
        masked = ctx.enter_context(nc.sbuf_tensor([U, U], F32))
        ones_sb = ctx.enter_context(nc.sbuf_tensor([U, 1], F32))
        s_sb = ctx.enter_context(nc.sbuf_tensor([128, 2], BF16))
        out_sb = ctx.enter_context(nc.sbuf_tensor([128, NTILES * MM_F], F32))
        psum = ctx.enter_context(nc.psum_tensor([128, 4096], F32))

        ctx.enter_context(nc.Block())
        block = nc.cur_block
        dma_c = ctx.enter_context(nc.semaphore("dma_c"))
        dma_x = [
            ctx.enter_context(nc.semaphore(f"dma_x{i}")) for i in range(NTILES)
        ]
        dma_out = ctx.enter_context(nc.semaphore("dma_out"))
        mm_sem = ctx.enter_context(nc.semaphore("mm_sem"))
        ve_sem = ctx.enter_context(nc.semaphore("ve_sem"))
        s_sem = ctx.enter_context(nc.semaphore("s_sem"))
        cpv_sem = ctx.enter_context(nc.semaphore("cpv_sem"))
        cps_sem = ctx.enter_context(nc.semaphore("cps_sem"))

        def psum_slot(g):
            q, bank = g // 16, g % 8
            return psum[32 * q : 32 * q + 2, bank * 512 : bank * 512 + MM_F]

        def out_slot(g):
            q, r = g // 16, g % 16
            return out_sb[32 * q : 32 * q + 2, r * MM_F : (r + 1) * MM_F]

        @block.sync
        def _(sync):
            sync.dma_start(out=conn_sb[:, :], in_=conn_d[:, :]).then_inc(dma_c, 16)
            for k in range(NTILES):
                if k >= NB:
                    # all 4 matmuls of tile k-NB done -> xb[k % NB] is free
                    sync.wait_ge(mm_sem, 2 + 4 * (k - NB) + 4)
                sync.dma_start(out=xb[k % NB][:, :], in_=x_d[k]).then_inc(
                    dma_x[k], 16
                )

        @block.gpsimd
        def _(gpsimd):
            # per-tile output drain: tile k's 4 matmul results are the
            # contiguous cols [(4k%16)*400, +1600) of pair q = k//4
            for k in range(NTILES):
                gpsimd.wait_ge(cpv_sem, 2 * (k + 1))
                gpsimd.wait_ge(cps_sem, 2 * (k + 1))
                q = k // 4
                c0 = (4 * k % 16) * MM_F
                gpsimd.dma_start(
                    out=out_d[2 * q : 2 * q + 2, c0 : c0 + 4 * MM_F],
                    in_=out_sb[32 * q : 32 * q + 2, c0 : c0 + 4 * MM_F],
                ).then_inc(dma_out, 16)
            gpsimd.wait_ge(dma_out, 16 * NTILES)

        @block.vector
        def _(vector):
            vector.memset(ones_sb[:, :], 1.0).then_inc(ve_sem)
            vector.memset(s_sb[:, :], 0.0).then_inc(ve_sem)
            vector.wait_ge(dma_c, 16)
            # masked = (conn > 0.1) * conn
            vector.scalar_tensor_tensor(
                out=masked[:, :],
                in0=conn_sb[:, :],
                scalar=THRESHOLD,
                in1=conn_sb[:, :],
                op0=mybir.AluOpType.is_gt,
                op1=mybir.AluOpType.mult,
            ).then_inc(ve_sem)
            for g in range(0, G_TOTAL, 2):
                vector.wait_ge(mm_sem, 3 + g)
                vector.tensor_copy(out=out_slot(g), in_=psum_slot(g)).then_inc(cpv_sem)

        @block.scalar
        def _(scalar):
            scalar.wait_ge(mm_sem, 2)
            # S[0:64, 0] = 3 * w ; S[64:128, 1] = 3 * w  (block diagonal)
            scalar.mul(s_sb[0:64, 0:1], psum[0:64, 0:1], STRENGTH).then_inc(s_sem)
            scalar.mul(s_sb[64:128, 1:2], psum[64:128, 0:1], STRENGTH).then_inc(s_sem)
            for g in range(1, G_TOTAL, 2):
                scalar.wait_ge(mm_sem, 3 + g)
                scalar.copy(out_slot(g), psum_slot(g)).then_inc(cps_sem)

        @block.tensor
        def _(tensor):
            tensor.wait_ge(ve_sem, 3)
            # w[j] = sum_i masked[i, j], materialized on partitions 0-63 and 64-127
            tensor.matmul(
                psum[0:64, 0:1], masked[:, :], ones_sb[:, :], start=True, stop=True
            ).then_inc(mm_sem)
            tensor.matmul(
                psum[64:128, 0:1],
                masked[:, :],
                ones_sb[:, :],
                start=True,
                stop=True,
                tile_position=(0, 64),
            ).then_inc(mm_sem)
            tensor.wait_ge(s_sem, 2)
            for k in range(NTILES):
                tensor.wait_ge(dma_x[k], 16)
                for m in range(MMS_PER_TILE):
                    g = 4 * k + m
                    if g >= 8:
                        gg = g - 8  # copy of the matmul that last used this bank
                        if gg % 2 == 0:
                            tensor.wait_ge(cpv_sem, gg // 2 + 1)
                        else:
                            tensor.wait_ge(cps_sem, gg // 2 + 1)
                    q = g // 16
                    tensor.matmul(
                        psum_slot(g),
                        s_sb[:, :],
                        xb[k % NB][:, m * MM_F : (m + 1) * MM_F],
                        start=True,
                        stop=True,
                        tile_position=(0, 32 * q),
                    ).then_inc(mm_sem)

    return nc


def shard_inputs(unit_outputs: np.ndarray, conn: np.ndarray):
    """Full inputs -> per-core in_maps with the tile layout the kernel expects.

    Shards are pre-rounded to bf16 (what the device matmul consumes anyway)
    so the HBM stream moves half the bytes.
    """
    import ml_dtypes

    conn = np.ascontiguousarray(conn, dtype=np.float32)
    in_maps = []
    for c in range(NCORES):
        xc = np.ascontiguousarray(
            unit_outputs[:, :, c * NS : (c + 1) * NS], dtype=np.float32
        ).reshape(U, FLAT)
        # [u, k, h, f] -> [k, (h u), f]
        v = xc.reshape(U, NTILES, 2, TILE_F)
        tiles = np.ascontiguousarray(
            v.transpose(1, 2, 0, 3).astype(ml_dtypes.bfloat16)
        ).reshape(NTILES, 128, TILE_F)
        in_maps.append({"x": tiles, "conn": conn})
    return in_maps


def unshard_output(results) -> np.ndarray:
    """Per-core [8, 6400] outputs -> full [T, N]."""
    final = np.empty((T, N), dtype=np.float32)
    for c in range(NCORES):
        r = np.asarray(results[c]["out"], dtype=np.float32)
        # rows 2q+h, cols (g%16)*400+c ; g = 16q + 4*r1 + r2 ; k = 4q+r1, m = r2
        arr = r.reshape(4, 2, 4, 4, MM_F)  # [q, h, r1, r2, c]
        flat = arr.transpose(0, 2, 1, 3, 4).reshape(FLAT)  # [k, h, m, c] order
        final[:, c * NS : (c + 1) * NS] = flat.reshape(T, NS)
    return final


_NC_CACHE = None


def kernel(unit_outputs: np.ndarray, conn: np.ndarray) -> np.ndarray:
    global _NC_CACHE
    if _NC_CACHE is None:
        _NC_CACHE = build_nc()
    in_maps = shard_inputs(unit_outputs, conn)
    res = run_bass_kernel_spmd(_NC_CACHE, in_maps, core_ids=list(range(NCORES)))
    return unshard_output(res.results)


if __name__ == "__main__":
    rng = np.random.default_rng(0)
    uo = rng.random((U, T, N), dtype=np.float32)
    cn = rng.random((U, U), dtype=np.float32)
    out = kernel(uo, cn)
    w = np.where(cn > THRESHOLD, cn, 0.0).sum(axis=0) * STRENGTH
    ref = np.einsum("j,jtn->tn", w, uo)
    err = np.abs(out - ref).max() / np.abs(ref).max()
    print("rel err:", err)
